# revision 7
# baseline (speedup 1.0000x reference)
"""InstantNGP hash-embedding kernel for trn2 (8 NeuronCores).

Data-parallel over the 1M points (sharding hint): each core processes its
batch shard for all 16 levels; the 67MB table stack is replicated per core
(uploaded to device HBM once and reused across the 4 program invocations).

Device program (SPMD, identical on all 8 cores), per 32768-point slice:
  - DVE integer/float pipeline: rel = (x+1)*recip, floor via round-nearest
    cast + is_gt fixup, clip, trilinear weights, and the spatial hash
    h = (ix ^ iy*p2 ^ iz*p3) & (2^19-1) computed exactly with products
    kept < 2^24 (the DVE int mult is fp32-based).
  - Gather: 8 corners x 16 levels via indirect DMA, 128 offsets (one per
    partition) per instruction, fully unrolled (hardware For_i loops
    deadlock with Pool-queue DMAs on this runtime, and InstDMAGatherAnt
    hangs, so the canonical unrolled form is used).
  - Interpolation: weighted 8-corner reduction on DVE, accumulated into a
    [*, 32] tile and stored contiguously.

Falls back to a pure-numpy host implementation if the device stack is
unavailable.
"""
import numpy as np

COORD_DIM = 3
GRID_MIN = -1.0
GRID_MAX = 1.0
N_LEVELS = 16
N_FEATS = 2
LOG2_T = 19
TABLE_SIZE = 2 ** LOG2_T
BASE_RES = 16
FINEST_RES = 512
BSZ = 1048576
N_CORES = 8
MASK = TABLE_SIZE - 1

_growth = np.exp((np.log(FINEST_RES) - np.log(BASE_RES)) / (N_LEVELS - 1))
RESOLUTIONS = [int(np.floor(BASE_RES * _growth ** i)) for i in range(N_LEVELS)]

P2 = 2654435761
P3 = 805459861
KY = P2 % TABLE_SIZE
KZ = P3 % TABLE_SIZE
K2Y = (32 * KY) % TABLE_SIZE
K2Z = (32 * KZ) % TABLE_SIZE

PTS_PER_CORE = BSZ // N_CORES        # 131072
LAST_DEVICE_NS = 0
CALLS = 4
PTS_PER_CALL = PTS_PER_CORE // CALLS  # 32768 per core per program run
N_PART = PTS_PER_CALL // 128          # 256 points per partition


def _build_program():
    from contextlib import ExitStack
    import concourse.bass as bass
    import concourse.bacc as bacc
    import concourse.tile as tile
    import concourse.mybir as mybir

    dt = mybir.dt
    Alu = mybir.AluOpType
    n = N_PART

    nc = bacc.Bacc("TRN2", target_bir_lowering=False)
    xin = nc.dram_tensor("xin", [PTS_PER_CALL, 3], dt.float32, kind="ExternalInput")
    emb = nc.dram_tensor("emb", [N_LEVELS * TABLE_SIZE, 2], dt.float32,
                         kind="ExternalInput")
    outp = nc.dram_tensor("outp", [PTS_PER_CALL, 2 * N_LEVELS], dt.float32,
                          kind="ExternalOutput")

    with tile.TileContext(nc) as tc, ExitStack() as ctx:
        pool = ctx.enter_context(tc.tile_pool(name="p", bufs=1))
        t = lambda shape, dtype, tag: pool.tile(shape, dtype, tag=tag, name=tag)

        x_sb = t([128, n, 3], dt.float32, "x")
        rel = t([128, n, 3], dt.float32, "rel")
        rf = t([128, n, 3], dt.float32, "rf")
        gtb = t([128, n, 3], dt.float32, "gtb")
        om = t([128, n, 3], dt.float32, "om")
        ii = t([128, n, 3], dt.int32, "ii")
        J1 = t([128, n], dt.int32, "J1")
        J2 = t([128, n], dt.int32, "J2")
        HX1 = t([128, n], dt.int32, "HX1")
        HY = t([128, n], dt.int32, "HY")
        HY1 = t([128, n], dt.int32, "HY1")
        HZ = t([128, n], dt.int32, "HZ")
        HZ1 = t([128, n], dt.int32, "HZ1")
        E = [t([128, n], dt.int32, f"E{k}") for k in range(4)]
        H8 = [t([128, n], dt.int32, f"H{k}") for k in range(8)]
        TXY = [t([128, n], dt.float32, f"T{k}") for k in range(4)]
        W8 = [t([128, n], dt.float32, f"W{k}") for k in range(8)]
        verts = [t([128, n, 2], dt.float32, f"V{k}") for k in range(8)]
        vsum = t([128, n, 2], dt.float32, "vsum")
        vtmp = t([128, n, 2], dt.float32, "vtmp")
        acc = t([128, n, 32], dt.float32, "acc")

        def hash_dim(ic_ap, K, K2, H, H1):
            nc.vector.tensor_scalar(out=J1[:], in0=ic_ap, scalar1=31,
                                    scalar2=None, op0=Alu.bitwise_and)
            nc.vector.tensor_scalar(out=J1[:], in0=J1[:], scalar1=K,
                                    scalar2=None, op0=Alu.mult)
            nc.vector.tensor_scalar(out=J1[:], in0=J1[:], scalar1=MASK,
                                    scalar2=None, op0=Alu.bitwise_and)
            nc.vector.tensor_scalar(out=J2[:], in0=ic_ap, scalar1=5,
                                    scalar2=None, op0=Alu.logical_shift_right)
            nc.vector.tensor_scalar(out=J2[:], in0=J2[:], scalar1=K2,
                                    scalar2=None, op0=Alu.mult)
            nc.vector.tensor_scalar(out=J2[:], in0=J2[:], scalar1=MASK,
                                    scalar2=None, op0=Alu.bitwise_and)
            nc.vector.tensor_tensor(out=H[:], in0=J1[:], in1=J2[:], op=Alu.add)
            nc.vector.tensor_scalar(out=H[:], in0=H[:], scalar1=MASK,
                                    scalar2=None, op0=Alu.bitwise_and)
            nc.vector.tensor_scalar(out=H1[:], in0=H[:], scalar1=K,
                                    scalar2=None, op0=Alu.add)
            nc.vector.tensor_scalar(out=H1[:], in0=H1[:], scalar1=MASK,
                                    scalar2=None, op0=Alu.bitwise_and)

        nc.sync.dma_start(
            x_sb[:].rearrange("p n d -> p (n d)"),
            xin[:, :].rearrange("(p n) d -> p (n d)", p=128))

        for lvl in range(N_LEVELS):
            res = RESOLUTIONS[lvl]
            grid_size = np.float32((GRID_MAX - GRID_MIN) / res)
            recip = float(np.float32(1.0 / float(grid_size)))

            nc.vector.tensor_scalar(out=rel[:], in0=x_sb[:],
                                    scalar1=float(-GRID_MIN), scalar2=recip,
                                    op0=Alu.add, op1=Alu.mult)
            nc.vector.tensor_copy(out=ii[:], in_=rel[:])
            nc.vector.tensor_copy(out=rf[:], in_=ii[:])
            nc.vector.tensor_tensor(out=gtb[:], in0=rf[:], in1=rel[:],
                                    op=Alu.is_gt)
            nc.vector.tensor_tensor(out=rf[:], in0=rf[:], in1=gtb[:],
                                    op=Alu.subtract)
            nc.vector.tensor_scalar(out=rf[:], in0=rf[:], scalar1=0.0,
                                    scalar2=float(res - 1), op0=Alu.max,
                                    op1=Alu.min)
            nc.vector.tensor_tensor(out=rel[:], in0=rel[:], in1=rf[:],
                                    op=Alu.subtract)
            nc.vector.tensor_scalar(out=om[:], in0=rel[:], scalar1=-1.0,
                                    scalar2=1.0, op0=Alu.mult, op1=Alu.add)
            nc.vector.tensor_copy(out=ii[:], in_=rf[:])

            hash_dim(ii[:, :, 1], KY, K2Y, HY, HY1)
            hash_dim(ii[:, :, 2], KZ, K2Z, HZ, HZ1)
            nc.vector.tensor_scalar(out=HX1[:], in0=ii[:, :, 0], scalar1=1,
                                    scalar2=None, op0=Alu.add)
            nc.vector.tensor_tensor(out=E[0][:], in0=ii[:, :, 0], in1=HY[:],
                                    op=Alu.bitwise_xor)
            nc.vector.tensor_tensor(out=E[1][:], in0=HX1[:], in1=HY[:],
                                    op=Alu.bitwise_xor)
            nc.vector.tensor_tensor(out=E[2][:], in0=ii[:, :, 0], in1=HY1[:],
                                    op=Alu.bitwise_xor)
            nc.vector.tensor_tensor(out=E[3][:], in0=HX1[:], in1=HY1[:],
                                    op=Alu.bitwise_xor)
            for k in range(8):
                nc.vector.tensor_tensor(out=H8[k][:], in0=E[k % 4][:],
                                        in1=(HZ if k < 4 else HZ1)[:],
                                        op=Alu.bitwise_xor)

            wx, wy, wz = rel[:, :, 0], rel[:, :, 1], rel[:, :, 2]
            ox, oy, oz = om[:, :, 0], om[:, :, 1], om[:, :, 2]
            nc.vector.tensor_tensor(out=TXY[0][:], in0=ox, in1=oy, op=Alu.mult)
            nc.vector.tensor_tensor(out=TXY[1][:], in0=wx, in1=oy, op=Alu.mult)
            nc.vector.tensor_tensor(out=TXY[2][:], in0=ox, in1=wy, op=Alu.mult)
            nc.vector.tensor_tensor(out=TXY[3][:], in0=wx, in1=wy, op=Alu.mult)
            for k in range(8):
                nc.vector.tensor_tensor(out=W8[k][:], in0=TXY[k % 4][:],
                                        in1=(oz if k < 4 else wz), op=Alu.mult)

            for s in range(n):
                for k in range(8):
                    nc.gpsimd.indirect_dma_start(
                        out=verts[k][:, s, :],
                        out_offset=None,
                        in_=emb[:, :],
                        in_offset=bass.IndirectOffsetOnAxis(
                            ap=H8[k][:, s:s + 1], axis=0),
                        element_offset=lvl * TABLE_SIZE * 2,
                    )

            for k in range(8):
                wb = W8[k][:].unsqueeze(2).to_broadcast([128, n, 2])
                if k == 0:
                    nc.vector.tensor_tensor(out=vsum[:], in0=verts[0][:],
                                            in1=wb, op=Alu.mult)
                else:
                    nc.vector.tensor_tensor(out=vtmp[:], in0=verts[k][:],
                                            in1=wb, op=Alu.mult)
                    nc.vector.tensor_tensor(out=vsum[:], in0=vsum[:],
                                            in1=vtmp[:], op=Alu.add)
            nc.vector.tensor_copy(out=acc[:, :, 2 * lvl:2 * lvl + 2],
                                  in_=vsum[:])

        nc.sync.dma_start(
            outp[:, :].rearrange("(p n) d -> p (n d)", p=128),
            acc[:].rearrange("p n d -> p (n d)"))
    nc.finalize()
    return nc


def _run_device(x, embeddings):
    import jax
    from jax.sharding import Mesh, PartitionSpec, NamedSharding
    from jax.experimental.shard_map import shard_map
    import concourse.mybir as mybir
    from concourse.bass2jax import (_bass_exec_p, install_neuronx_cc_hook,
                                    partition_id_tensor)

    nc = _build_program()
    install_neuronx_cc_hook()

    partition_name = nc.partition_id_tensor.name if nc.partition_id_tensor else None
    in_names, out_names, out_avals, zero_shapes = [], [], [], []
    for alloc in nc.m.functions[0].allocations:
        if not isinstance(alloc, mybir.MemoryLocationSet):
            continue
        name = alloc.memorylocations[0].name
        if alloc.kind == "ExternalInput":
            if name != partition_name:
                in_names.append(name)
        elif alloc.kind == "ExternalOutput":
            out_names.append(name)
            shape = tuple(alloc.tensor_shape)
            dtype = mybir.dt.np(alloc.dtype)
            out_avals.append(jax.core.ShapedArray(shape, dtype))
            zero_shapes.append((shape, dtype))
    n_params = len(in_names)
    n_outs = len(out_avals)
    all_in_names = list(in_names) + list(out_names)
    if partition_name is not None:
        all_in_names.append(partition_name)
    donate = tuple(range(n_params, n_params + n_outs))

    def _body(*args):
        operands = list(args)
        if partition_name is not None:
            operands.append(partition_id_tensor())
        outs = _bass_exec_p.bind(
            *operands,
            out_avals=tuple(out_avals),
            in_names=tuple(all_in_names),
            out_names=tuple(out_names),
            lowering_input_output_aliases=(),
            sim_require_finite=True,
            sim_require_nnan=True,
            nc=nc,
        )
        return tuple(outs)

    devices = jax.devices()[:N_CORES]
    mesh = Mesh(np.asarray(devices), ("core",))
    in_specs = (PartitionSpec("core"),) * (n_params + n_outs)
    out_specs = (PartitionSpec("core"),) * n_outs
    jitted = jax.jit(
        shard_map(_body, mesh=mesh, in_specs=in_specs, out_specs=out_specs,
                  check_rep=False),
        donate_argnums=donate, keep_unused=True)

    # Upload the replicated table stack once; reuse across the CALLS runs.
    embf = np.ascontiguousarray(embeddings.reshape(N_LEVELS * TABLE_SIZE, 2))
    sharding = NamedSharding(mesh, PartitionSpec("core"))
    emb_big = np.broadcast_to(embf, (N_CORES,) + embf.shape).reshape(
        N_CORES * embf.shape[0], embf.shape[1])
    emb_dev = jax.device_put(emb_big, sharding)

    x = np.ascontiguousarray(x, dtype=np.float32)
    # core-major shards, each split into CALLS slices
    xs = x.reshape(N_CORES, CALLS, PTS_PER_CALL, 3)

    # Donated output buffers are pre-zeroed ON DEVICE (the kernel writes
    # every element, but the custom-call lowering allocates results by
    # donating inputs) — avoids shipping 33.5MB of zeros per call.
    import jax.numpy as jnp
    zeros_fn = jax.jit(
        lambda: tuple(jnp.zeros((N_CORES * s[0],) + s[1:], d)
                      for s, d in zero_shapes),
        out_shardings=(sharding,) * n_outs)

    # Warm the executable (jit/NEFF load) so the timed loop below measures
    # steady-state execution of the production calls.
    warm_args = []
    for name in in_names:
        if name == "xin":
            warm_args.append(np.zeros((N_CORES * PTS_PER_CALL, 3), np.float32))
        elif name == "emb":
            warm_args.append(emb_dev)
    jax.block_until_ready(jitted(*warm_args, *zeros_fn()))

    x_calls = [np.ascontiguousarray(xs[:, call].reshape(
        N_CORES * PTS_PER_CALL, 3)) for call in range(CALLS)]

    import time as _time
    _dev_t0 = _time.perf_counter()
    # Launch all calls asynchronously; device queues keep them in order and
    # output downloads overlap the next call's execution.
    pending = []
    for call in range(CALLS):
        args = []
        for name in in_names:
            if name == "xin":
                args.append(x_calls[call])
            elif name == "emb":
                args.append(emb_dev)
            else:
                raise KeyError(name)
        pending.append(jitted(*args, *zeros_fn()))
    out_slices = [np.asarray(outs[out_names.index("outp")]) for outs in pending]
    global LAST_DEVICE_NS
    LAST_DEVICE_NS = int((_time.perf_counter() - _dev_t0) * 1e9)

    # out_slices[call]: (N_CORES*PTS_PER_CALL, 32) core-major
    full = np.empty((BSZ, 2 * N_LEVELS), np.float32)
    for call in range(CALLS):
        o = out_slices[call].reshape(N_CORES, PTS_PER_CALL, 2 * N_LEVELS)
        for c in range(N_CORES):
            b0 = c * PTS_PER_CORE + call * PTS_PER_CALL
            full[b0:b0 + PTS_PER_CALL] = o[c]
    return full


def _host_kernel(x, embeddings):
    PRIMES = np.array([1, P2, P3], dtype=np.uint32)
    OFFSETS = np.array([[(k >> d) & 1 for d in range(COORD_DIM)]
                        for k in range(2 ** COORD_DIM)], dtype=np.uint32)
    s = x - np.float32(GRID_MIN)
    outs = []
    for lvl in range(N_LEVELS):
        res = RESOLUTIONS[lvl]
        grid_size = np.float32((GRID_MAX - GRID_MIN) / res)
        recip = np.float32(1.0 / float(grid_size))
        rel = s * recip
        idx0 = np.clip(np.floor(rel), 0, res - 1)
        w = rel - idx0
        vidx = idx0.astype(np.uint32)[:, None, :] + OFFSETS[None]
        vp = vidx * PRIMES[None, None, :]
        h = (vp[..., 0] ^ vp[..., 1] ^ vp[..., 2]) & np.uint32(MASK)
        verts = embeddings[lvl][h.astype(np.int64)]
        wb = w[:, None, :].astype(np.float32)
        terms = np.where(OFFSETS[None] == 1, wb, np.float32(1.0) - wb)
        wts = (terms[..., 0] * terms[..., 1] * terms[..., 2]).astype(np.float32)
        out = np.zeros((x.shape[0], N_FEATS), np.float32)
        for v in range(8):
            out += wts[:, v, None] * verts[:, v, :]
        outs.append(out)
    return np.concatenate(outs, axis=-1)


def kernel(x, embeddings):
    x = np.asarray(x, dtype=np.float32)
    emb = np.asarray(embeddings, dtype=np.float32)
    try:
        return _run_device(x, emb)
    except Exception:
        import traceback
        traceback.print_exc()
        return _host_kernel(x, emb)


# revision 8
# speedup vs baseline: 1.2381x; 1.2381x over previous
"""InstantNGP hash-embedding kernel for trn2 (8 NeuronCores).

Data-parallel over the 1M points (sharding hint): each core processes its
batch shard for all 16 levels; the 67MB table stack is replicated per core
(uploaded to device HBM once and reused across the 4 program invocations).

Device program (SPMD, identical on all 8 cores), per 32768-point slice:
  - DVE integer/float pipeline: rel = (x+1)*recip, floor via round-nearest
    cast + is_gt fixup, clip, trilinear weights, and the spatial hash
    h = (ix ^ iy*p2 ^ iz*p3) & (2^19-1) computed exactly with products
    kept < 2^24 (the DVE int mult is fp32-based).
  - Gather: 8 corners x 16 levels via indirect DMA, 128 offsets (one per
    partition) per instruction, fully unrolled (hardware For_i loops
    deadlock with Pool-queue DMAs on this runtime, and InstDMAGatherAnt
    hangs, so the canonical unrolled form is used).
  - Interpolation: weighted 8-corner reduction on DVE, accumulated into a
    [*, 32] tile and stored contiguously.

Falls back to a pure-numpy host implementation if the device stack is
unavailable.
"""
import numpy as np

COORD_DIM = 3
GRID_MIN = -1.0
GRID_MAX = 1.0
N_LEVELS = 16
N_FEATS = 2
LOG2_T = 19
TABLE_SIZE = 2 ** LOG2_T
BASE_RES = 16
FINEST_RES = 512
BSZ = 1048576
N_CORES = 8
MASK = TABLE_SIZE - 1

_growth = np.exp((np.log(FINEST_RES) - np.log(BASE_RES)) / (N_LEVELS - 1))
RESOLUTIONS = [int(np.floor(BASE_RES * _growth ** i)) for i in range(N_LEVELS)]

P2 = 2654435761
P3 = 805459861
KY = P2 % TABLE_SIZE
KZ = P3 % TABLE_SIZE
K2Y = (32 * KY) % TABLE_SIZE
K2Z = (32 * KZ) % TABLE_SIZE

PTS_PER_CORE = BSZ // N_CORES        # 131072
LAST_DEVICE_NS = 0
CALLS = 2
PTS_PER_CALL = PTS_PER_CORE // CALLS  # 32768 per core per program run
N_PART = PTS_PER_CALL // 128          # 256 points per partition


def _build_program():
    from contextlib import ExitStack
    import concourse.bass as bass
    import concourse.bacc as bacc
    import concourse.tile as tile
    import concourse.mybir as mybir

    dt = mybir.dt
    Alu = mybir.AluOpType
    n = N_PART

    nc = bacc.Bacc("TRN2", target_bir_lowering=False)
    xin = nc.dram_tensor("xin", [PTS_PER_CALL, 3], dt.float32, kind="ExternalInput")
    emb = nc.dram_tensor("emb", [N_LEVELS * TABLE_SIZE, 2], dt.float32,
                         kind="ExternalInput")
    outp = nc.dram_tensor("outp", [PTS_PER_CALL, 2 * N_LEVELS], dt.float32,
                          kind="ExternalOutput")

    with tile.TileContext(nc) as tc, ExitStack() as ctx:
        pool = ctx.enter_context(tc.tile_pool(name="p", bufs=1))
        t = lambda shape, dtype, tag: pool.tile(shape, dtype, tag=tag, name=tag)

        x_sb = t([128, n, 3], dt.float32, "x")
        rel = t([128, n, 3], dt.float32, "rel")
        rf = t([128, n, 3], dt.float32, "rf")
        gtb = t([128, n, 3], dt.float32, "gtb")
        om = t([128, n, 3], dt.float32, "om")
        ii = t([128, n, 3], dt.int32, "ii")
        J1 = t([128, n], dt.int32, "J1")
        J2 = t([128, n], dt.int32, "J2")
        HX1 = t([128, n], dt.int32, "HX1")
        HY = t([128, n], dt.int32, "HY")
        HY1 = t([128, n], dt.int32, "HY1")
        HZ = t([128, n], dt.int32, "HZ")
        HZ1 = t([128, n], dt.int32, "HZ1")
        E = [t([128, n], dt.int32, f"E{k}") for k in range(4)]
        H8 = [t([128, n], dt.int32, f"H{k}") for k in range(8)]
        TXY = [t([128, n], dt.float32, f"T{k}") for k in range(4)]
        W8 = [t([128, n], dt.float32, f"W{k}") for k in range(8)]
        verts = [t([128, n, 2], dt.float32, f"V{k}") for k in range(8)]
        vsum = t([128, n, 2], dt.float32, "vsum")
        vtmp = t([128, n, 2], dt.float32, "vtmp")
        acc = t([128, n, 32], dt.float32, "acc")

        def hash_dim(ic_ap, K, K2, H, H1):
            nc.vector.tensor_scalar(out=J1[:], in0=ic_ap, scalar1=31,
                                    scalar2=None, op0=Alu.bitwise_and)
            nc.vector.tensor_scalar(out=J1[:], in0=J1[:], scalar1=K,
                                    scalar2=None, op0=Alu.mult)
            nc.vector.tensor_scalar(out=J1[:], in0=J1[:], scalar1=MASK,
                                    scalar2=None, op0=Alu.bitwise_and)
            nc.vector.tensor_scalar(out=J2[:], in0=ic_ap, scalar1=5,
                                    scalar2=None, op0=Alu.logical_shift_right)
            nc.vector.tensor_scalar(out=J2[:], in0=J2[:], scalar1=K2,
                                    scalar2=None, op0=Alu.mult)
            nc.vector.tensor_scalar(out=J2[:], in0=J2[:], scalar1=MASK,
                                    scalar2=None, op0=Alu.bitwise_and)
            nc.vector.tensor_tensor(out=H[:], in0=J1[:], in1=J2[:], op=Alu.add)
            nc.vector.tensor_scalar(out=H[:], in0=H[:], scalar1=MASK,
                                    scalar2=None, op0=Alu.bitwise_and)
            nc.vector.tensor_scalar(out=H1[:], in0=H[:], scalar1=K,
                                    scalar2=None, op0=Alu.add)
            nc.vector.tensor_scalar(out=H1[:], in0=H1[:], scalar1=MASK,
                                    scalar2=None, op0=Alu.bitwise_and)

        nc.sync.dma_start(
            x_sb[:].rearrange("p n d -> p (n d)"),
            xin[:, :].rearrange("(p n) d -> p (n d)", p=128))

        for lvl in range(N_LEVELS):
            res = RESOLUTIONS[lvl]
            grid_size = np.float32((GRID_MAX - GRID_MIN) / res)
            recip = float(np.float32(1.0 / float(grid_size)))

            nc.vector.tensor_scalar(out=rel[:], in0=x_sb[:],
                                    scalar1=float(-GRID_MIN), scalar2=recip,
                                    op0=Alu.add, op1=Alu.mult)
            nc.vector.tensor_copy(out=ii[:], in_=rel[:])
            nc.vector.tensor_copy(out=rf[:], in_=ii[:])
            nc.vector.tensor_tensor(out=gtb[:], in0=rf[:], in1=rel[:],
                                    op=Alu.is_gt)
            nc.vector.tensor_tensor(out=rf[:], in0=rf[:], in1=gtb[:],
                                    op=Alu.subtract)
            nc.vector.tensor_scalar(out=rf[:], in0=rf[:], scalar1=0.0,
                                    scalar2=float(res - 1), op0=Alu.max,
                                    op1=Alu.min)
            nc.vector.tensor_tensor(out=rel[:], in0=rel[:], in1=rf[:],
                                    op=Alu.subtract)
            nc.vector.tensor_scalar(out=om[:], in0=rel[:], scalar1=-1.0,
                                    scalar2=1.0, op0=Alu.mult, op1=Alu.add)
            nc.vector.tensor_copy(out=ii[:], in_=rf[:])

            hash_dim(ii[:, :, 1], KY, K2Y, HY, HY1)
            hash_dim(ii[:, :, 2], KZ, K2Z, HZ, HZ1)
            nc.vector.tensor_scalar(out=HX1[:], in0=ii[:, :, 0], scalar1=1,
                                    scalar2=None, op0=Alu.add)
            nc.vector.tensor_tensor(out=E[0][:], in0=ii[:, :, 0], in1=HY[:],
                                    op=Alu.bitwise_xor)
            nc.vector.tensor_tensor(out=E[1][:], in0=HX1[:], in1=HY[:],
                                    op=Alu.bitwise_xor)
            nc.vector.tensor_tensor(out=E[2][:], in0=ii[:, :, 0], in1=HY1[:],
                                    op=Alu.bitwise_xor)
            nc.vector.tensor_tensor(out=E[3][:], in0=HX1[:], in1=HY1[:],
                                    op=Alu.bitwise_xor)
            for k in range(8):
                nc.vector.tensor_tensor(out=H8[k][:], in0=E[k % 4][:],
                                        in1=(HZ if k < 4 else HZ1)[:],
                                        op=Alu.bitwise_xor)

            wx, wy, wz = rel[:, :, 0], rel[:, :, 1], rel[:, :, 2]
            ox, oy, oz = om[:, :, 0], om[:, :, 1], om[:, :, 2]
            nc.vector.tensor_tensor(out=TXY[0][:], in0=ox, in1=oy, op=Alu.mult)
            nc.vector.tensor_tensor(out=TXY[1][:], in0=wx, in1=oy, op=Alu.mult)
            nc.vector.tensor_tensor(out=TXY[2][:], in0=ox, in1=wy, op=Alu.mult)
            nc.vector.tensor_tensor(out=TXY[3][:], in0=wx, in1=wy, op=Alu.mult)
            for k in range(8):
                nc.vector.tensor_tensor(out=W8[k][:], in0=TXY[k % 4][:],
                                        in1=(oz if k < 4 else wz), op=Alu.mult)

            for s in range(n):
                for k in range(8):
                    nc.gpsimd.indirect_dma_start(
                        out=verts[k][:, s, :],
                        out_offset=None,
                        in_=emb[:, :],
                        in_offset=bass.IndirectOffsetOnAxis(
                            ap=H8[k][:, s:s + 1], axis=0),
                        element_offset=lvl * TABLE_SIZE * 2,
                    )

            for k in range(8):
                wb = W8[k][:].unsqueeze(2).to_broadcast([128, n, 2])
                if k == 0:
                    nc.vector.tensor_tensor(out=vsum[:], in0=verts[0][:],
                                            in1=wb, op=Alu.mult)
                else:
                    nc.vector.tensor_tensor(out=vtmp[:], in0=verts[k][:],
                                            in1=wb, op=Alu.mult)
                    nc.vector.tensor_tensor(out=vsum[:], in0=vsum[:],
                                            in1=vtmp[:], op=Alu.add)
            nc.vector.tensor_copy(out=acc[:, :, 2 * lvl:2 * lvl + 2],
                                  in_=vsum[:])

        nc.sync.dma_start(
            outp[:, :].rearrange("(p n) d -> p (n d)", p=128),
            acc[:].rearrange("p n d -> p (n d)"))
    nc.finalize()
    return nc


def _run_device(x, embeddings):
    import jax
    from jax.sharding import Mesh, PartitionSpec, NamedSharding
    from jax.experimental.shard_map import shard_map
    import concourse.mybir as mybir
    from concourse.bass2jax import (_bass_exec_p, install_neuronx_cc_hook,
                                    partition_id_tensor)

    nc = _build_program()
    install_neuronx_cc_hook()

    partition_name = nc.partition_id_tensor.name if nc.partition_id_tensor else None
    in_names, out_names, out_avals, zero_shapes = [], [], [], []
    for alloc in nc.m.functions[0].allocations:
        if not isinstance(alloc, mybir.MemoryLocationSet):
            continue
        name = alloc.memorylocations[0].name
        if alloc.kind == "ExternalInput":
            if name != partition_name:
                in_names.append(name)
        elif alloc.kind == "ExternalOutput":
            out_names.append(name)
            shape = tuple(alloc.tensor_shape)
            dtype = mybir.dt.np(alloc.dtype)
            out_avals.append(jax.core.ShapedArray(shape, dtype))
            zero_shapes.append((shape, dtype))
    n_params = len(in_names)
    n_outs = len(out_avals)
    all_in_names = list(in_names) + list(out_names)
    if partition_name is not None:
        all_in_names.append(partition_name)
    donate = tuple(range(n_params, n_params + n_outs))

    def _body(*args):
        operands = list(args)
        if partition_name is not None:
            operands.append(partition_id_tensor())
        outs = _bass_exec_p.bind(
            *operands,
            out_avals=tuple(out_avals),
            in_names=tuple(all_in_names),
            out_names=tuple(out_names),
            lowering_input_output_aliases=(),
            sim_require_finite=True,
            sim_require_nnan=True,
            nc=nc,
        )
        return tuple(outs)

    devices = jax.devices()[:N_CORES]
    mesh = Mesh(np.asarray(devices), ("core",))
    in_specs = (PartitionSpec("core"),) * (n_params + n_outs)
    out_specs = (PartitionSpec("core"),) * n_outs
    jitted = jax.jit(
        shard_map(_body, mesh=mesh, in_specs=in_specs, out_specs=out_specs,
                  check_rep=False),
        donate_argnums=donate, keep_unused=True)

    # Upload the replicated table stack once; reuse across the CALLS runs.
    embf = np.ascontiguousarray(embeddings.reshape(N_LEVELS * TABLE_SIZE, 2))
    sharding = NamedSharding(mesh, PartitionSpec("core"))
    emb_big = np.broadcast_to(embf, (N_CORES,) + embf.shape).reshape(
        N_CORES * embf.shape[0], embf.shape[1])
    emb_dev = jax.device_put(emb_big, sharding)

    x = np.ascontiguousarray(x, dtype=np.float32)
    # core-major shards, each split into CALLS slices
    xs = x.reshape(N_CORES, CALLS, PTS_PER_CALL, 3)

    # Donated output buffers are pre-zeroed ON DEVICE (the kernel writes
    # every element, but the custom-call lowering allocates results by
    # donating inputs) — avoids shipping 33.5MB of zeros per call.
    import jax.numpy as jnp
    zeros_fn = jax.jit(
        lambda: tuple(jnp.zeros((N_CORES * s[0],) + s[1:], d)
                      for s, d in zero_shapes),
        out_shardings=(sharding,) * n_outs)

    # Warm the executable (jit/NEFF load) so the timed loop below measures
    # steady-state execution of the production calls.
    warm_args = []
    for name in in_names:
        if name == "xin":
            warm_args.append(np.zeros((N_CORES * PTS_PER_CALL, 3), np.float32))
        elif name == "emb":
            warm_args.append(emb_dev)
    jax.block_until_ready(jitted(*warm_args, *zeros_fn()))

    x_calls = [np.ascontiguousarray(xs[:, call].reshape(
        N_CORES * PTS_PER_CALL, 3)) for call in range(CALLS)]

    import time as _time
    _dev_t0 = _time.perf_counter()
    # Launch all calls asynchronously; device queues keep them in order and
    # output downloads overlap the next call's execution.
    pending = []
    for call in range(CALLS):
        args = []
        for name in in_names:
            if name == "xin":
                args.append(x_calls[call])
            elif name == "emb":
                args.append(emb_dev)
            else:
                raise KeyError(name)
        pending.append(jitted(*args, *zeros_fn()))
    out_slices = [np.asarray(outs[out_names.index("outp")]) for outs in pending]
    global LAST_DEVICE_NS
    LAST_DEVICE_NS = int((_time.perf_counter() - _dev_t0) * 1e9)

    # out_slices[call]: (N_CORES*PTS_PER_CALL, 32) core-major
    full = np.empty((BSZ, 2 * N_LEVELS), np.float32)
    for call in range(CALLS):
        o = out_slices[call].reshape(N_CORES, PTS_PER_CALL, 2 * N_LEVELS)
        for c in range(N_CORES):
            b0 = c * PTS_PER_CORE + call * PTS_PER_CALL
            full[b0:b0 + PTS_PER_CALL] = o[c]
    return full


def _host_kernel(x, embeddings):
    PRIMES = np.array([1, P2, P3], dtype=np.uint32)
    OFFSETS = np.array([[(k >> d) & 1 for d in range(COORD_DIM)]
                        for k in range(2 ** COORD_DIM)], dtype=np.uint32)
    s = x - np.float32(GRID_MIN)
    outs = []
    for lvl in range(N_LEVELS):
        res = RESOLUTIONS[lvl]
        grid_size = np.float32((GRID_MAX - GRID_MIN) / res)
        recip = np.float32(1.0 / float(grid_size))
        rel = s * recip
        idx0 = np.clip(np.floor(rel), 0, res - 1)
        w = rel - idx0
        vidx = idx0.astype(np.uint32)[:, None, :] + OFFSETS[None]
        vp = vidx * PRIMES[None, None, :]
        h = (vp[..., 0] ^ vp[..., 1] ^ vp[..., 2]) & np.uint32(MASK)
        verts = embeddings[lvl][h.astype(np.int64)]
        wb = w[:, None, :].astype(np.float32)
        terms = np.where(OFFSETS[None] == 1, wb, np.float32(1.0) - wb)
        wts = (terms[..., 0] * terms[..., 1] * terms[..., 2]).astype(np.float32)
        out = np.zeros((x.shape[0], N_FEATS), np.float32)
        for v in range(8):
            out += wts[:, v, None] * verts[:, v, :]
        outs.append(out)
    return np.concatenate(outs, axis=-1)


def kernel(x, embeddings):
    x = np.asarray(x, dtype=np.float32)
    emb = np.asarray(embeddings, dtype=np.float32)
    try:
        return _run_device(x, emb)
    except Exception:
        import traceback
        traceback.print_exc()
        return _host_kernel(x, emb)
